# revision 7
# baseline (speedup 1.0000x reference)
"""Causal self-attention with RoPE, sharded over 8 TRN2 NeuronCores.

Sharding: core = (batch b, head-group hg). Cores 0-3 -> batch 0, cores 4-7 ->
batch 1; head-group hg = core % 4 owns heads [3*hg, 3*hg+3). Each core computes
its heads' attention and a partial output projection (w_proj column-slice);
the host sums the 4 partials per batch (the row-sharded projection's
all-reduce, done on host since full outputs are gathered anyway).

Per-core kernel (all fp32; matmuls run as float32r for full PE rate):
  - QKV in [feature, token] layout: out = wT.T @ xT (host pre-transposes).
    Feature tiles packed so each head's q and k share a partition half:
    ft0=[q0|q1] ft1=[k0|k1] ft2=[q2|pad] ft3=[k2|pad] ft4=[v0|v1] ft5=[v2|pad]
  - RoPE: rotate_half as a permutation matmul (p2t), combined on VectorE.
  - Attention in scores-transposed layout [keys, queries]: probs^T = exp(K^T.T
    @ Q^T * 0.125), causal mask via gpsimd affine_select (zero after exp),
    PV as out^T = (V|1).T @ probs^T accumulating over key tiles; the ones
    column yields the softmax denominator for free.
  - Division by denom folded into the PSUM->SBUF move (tensor_mul by
    partition-broadcast reciprocal).
  - Projection: partial out^T = wpT.T @ attn^T, DMA'd out as (768, 2048).
"""

import numpy as np

import concourse.bass as bass
import concourse.bacc as bacc
import concourse.tile as tile
from concourse import mybir
from concourse.bass_utils import run_bass_kernel_spmd

B, T, C, H = 2, 2048, 768, 12
D = C // H  # 64
ROPE_THETA = 10000.0
NCORES = 8
HPC = 3             # heads per core
FPAD = 768          # 6 x 128 padded qkv feature rows
QB = 512            # query block (free dim of scores^T tiles)
KT = 128            # key tile (partition dim of scores^T tiles)

F32 = mybir.dt.float32
F32R = mybir.dt.float32r

# (feature-tile, half) of each head's q / k block in the packed layout
Q_POS = {0: (0, 0), 1: (0, 1), 2: (2, 0)}
K_POS = {0: (1, 0), 1: (1, 1), 2: (3, 0)}
V_POS = {0: (4, 0), 1: (4, 1), 2: (5, 0)}


def _build_nc(t_len=T):
    nc = bacc.Bacc("TRN2", target_bir_lowering=False, debug=False)

    xT_d = nc.dram_tensor("xT", [C, t_len], F32, kind="ExternalInput")
    wT_d = nc.dram_tensor("wT", [C, FPAD], F32, kind="ExternalInput")
    wpT_d = nc.dram_tensor("wpT", [HPC * D, C], F32, kind="ExternalInput")
    cos_d = nc.dram_tensor("cosT", [128, t_len], F32, kind="ExternalInput")
    sin_d = nc.dram_tensor("sinT", [128, t_len], F32, kind="ExternalInput")
    p2t_d = nc.dram_tensor("p2t", [128, 128], F32, kind="ExternalInput")
    id_d = nc.dram_tensor("ident", [128, D], F32, kind="ExternalInput")
    outT_d = nc.dram_tensor("outT", [C, t_len], F32, kind="ExternalOutput")

    with tile.TileContext(nc) as tc:
        _body(tc, t_len, xT_d, wT_d, wpT_d, cos_d, sin_d, p2t_d, id_d, outT_d)
    nc.compile()
    return nc


def _body(tc, t_len, xT_d, wT_d, wpT_d, cos_d, sin_d, p2t_d, id_d, outT_d):
    nc = tc.nc
    T = t_len
    NQB = T // QB
    NKT = T // KT
    NCT = C // 128   # 6 contraction tiles over channels
    NFT = FPAD // 128  # 6 qkv feature tiles
    JPB = QB // KT   # key tiles per token block (4)

    with (
        tc.tile_pool(name="singles", bufs=1) as singles,
        tc.tile_pool(name="sb_x", bufs=2) as sb_x,
        tc.tile_pool(name="ps_acc", bufs=2, space="PSUM") as ps_acc,
        tc.tile_pool(name="ps_tmp", bufs=4, space="PSUM") as ps_tmp,
        tc.tile_pool(name="sb_probs", bufs=4) as sb_probs,
        tc.tile_pool(name="sb_raw", bufs=3) as sb_raw,
        tc.tile_pool(name="sb_tmp", bufs=3) as sb_tmp,
        tc.tile_pool(name="sb_out", bufs=3) as sb_out,
        tc.tile_pool(name="sb_rcp", bufs=2) as sb_rcp,
    ):
        # ---- persistent SBUF tensors -------------------------------------
        wT = singles.tile([128, NCT, FPAD], F32, tag="wT")
        wp0 = singles.tile([128, C], F32, tag="wp0")
        wp1 = singles.tile([64, C], F32, tag="wp1")
        cosc = singles.tile([128, T], F32, tag="cosc")
        sinc = singles.tile([128, T], F32, tag="sinc")
        p2t = singles.tile([128, 128], F32, tag="p2t")
        ident = singles.tile([128, D], F32, tag="ident")
        qkrot = singles.tile([128, 4, T], F32, tag="qkrot")
        va = singles.tile([128, NKT * HPC, D + 1], F32, tag="va")
        at01 = singles.tile([128, T], F32, tag="at01")  # heads 0,1
        at2 = singles.tile([64, T], F32, tag="at2")     # head 2

        wT_v = wT_d.ap().rearrange("(a p) f -> p a f", p=128)
        for a in range(NCT):
            nc.sync.dma_start(out=wT[:, a, :].bitcast(F32R), in_=wT_v[:, a, :].bitcast(F32R))
        nc.sync.dma_start(out=wp0.bitcast(F32R), in_=wpT_d.ap()[0:128, :].bitcast(F32R))
        nc.sync.dma_start(out=wp1.bitcast(F32R), in_=wpT_d.ap()[128:192, :].bitcast(F32R))
        nc.sync.dma_start(out=cosc, in_=cos_d.ap())
        nc.sync.dma_start(out=sinc, in_=sin_d.ap())
        nc.sync.dma_start(out=p2t.bitcast(F32R), in_=p2t_d.ap().bitcast(F32R))
        nc.sync.dma_start(out=ident, in_=id_d.ap())

        # ones column of the augmented V tiles (softmax denominator trick);
        # memset can't emit fp32r, so round via a DVE copy
        ones = singles.tile([128, NKT * HPC], F32, tag="ones")
        nc.vector.memset(ones, 1.0)
        nc.vector.tensor_copy(va[:, :, D : D + 1].bitcast(F32R), ones)

        xT_v = xT_d.ap().rearrange("(a p) t -> p a t", p=128)

        # ---- QKV projection + RoPE + V transpose -------------------------
        for tb in range(NQB):
            ts = slice(tb * QB, (tb + 1) * QB)
            xtb = sb_x.tile([128, NCT, QB], F32, tag="xtb")
            for ct in range(NCT):
                nc.sync.dma_start(out=xtb[:, ct, :].bitcast(F32R), in_=xT_v[:, ct, ts].bitcast(F32R))
            for ft in range(NFT):
                acc = ps_acc.tile([128, QB], F32, tag="ps_acc")
                for ct in range(NCT):
                    nc.tensor.matmul(
                        acc,
                        wT[:, ct, ft * 128 : (ft + 1) * 128].bitcast(F32R),
                        xtb[:, ct, :].bitcast(F32R),
                        start=(ct == 0),
                        stop=(ct == NCT - 1),
                    )
                raw = sb_raw.tile([128, QB], F32, tag="raw")
                nc.vector.tensor_copy(raw.bitcast(F32R), acc)
                if ft < 4:
                    # q|k tile: rotate-half matmul, combine with sin/cos
                    rh = ps_tmp.tile([128, QB], F32, tag="ps_tmp")
                    nc.tensor.matmul(
                        rh, p2t.bitcast(F32R), raw.bitcast(F32R),
                        start=True, stop=True,
                    )
                    tmp = sb_tmp.tile([128, QB], F32, tag="tmp")
                    nc.vector.tensor_mul(tmp, rh, sinc[:, ts])
                    nc.vector.tensor_mul(
                        qkrot[:, ft, ts].bitcast(F32R), raw, cosc[:, ts]
                    )
                    nc.vector.tensor_add(
                        qkrot[:, ft, ts].bitcast(F32R), qkrot[:, ft, ts], tmp
                    )
                else:
                    # v tile: transpose each head-half into [keys, D] layout
                    for half in range(2):
                        hv = (ft - 4) * 2 + half
                        if hv >= HPC:
                            continue
                        rs = slice(half * 64, half * 64 + 64)
                        for j in range(JPB):
                            kt = tb * JPB + j
                            tp = ps_tmp.tile([128, D], F32, tag="ps_tmp")
                            nc.tensor.transpose(
                                tp,
                                raw[rs, j * KT : (j + 1) * KT],
                                ident[rs, :],
                            )
                            nc.vector.tensor_copy(
                                va[:, kt * HPC + hv, 0:D].bitcast(F32R), tp
                            )

        # ---- attention ---------------------------------------------------
        def qk_ap(pos, ts):
            ti, half = pos
            return qkrot[half * 64 : half * 64 + 64, ti, ts]

        for h in range(HPC):
            for qb in range(NQB):
                qs = slice(qb * QB, (qb + 1) * QB)
                nkt = (qb + 1) * JPB  # key tiles in causal range
                pv = ps_acc.tile([65, QB], F32, tag="ps_acc")
                for kt in range(nkt):
                    sc = ps_tmp.tile([128, QB], F32, tag="ps_tmp")
                    nc.tensor.matmul(
                        sc,
                        qk_ap(K_POS[h], slice(kt * KT, (kt + 1) * KT))
                        .bitcast(F32R),
                        qk_ap(Q_POS[h], qs).bitcast(F32R),
                        start=True, stop=True,
                    )
                    probs = sb_probs.tile([128, QB], F32, tag="probs")
                    nc.scalar.activation(
                        probs.bitcast(F32R), sc,
                        mybir.ActivationFunctionType.Exp,
                        scale=float(1.0 / np.sqrt(D)),
                    )
                    base = qb * QB - kt * KT
                    if base < KT:  # diagonal tile: zero the masked region
                        nc.gpsimd.affine_select(
                            out=probs.bitcast(F32R), in_=probs.bitcast(F32R),
                            compare_op=mybir.AluOpType.is_ge,
                            fill=0.0, base=base,
                            pattern=[[1, QB]], channel_multiplier=-1,
                        )
                    nc.tensor.matmul(
                        pv,
                        va[:, kt * HPC + h, :].bitcast(F32R),
                        probs.bitcast(F32R),
                        start=(kt == 0),
                        stop=(kt == nkt - 1),
                    )
                rcp = sb_rcp.tile([1, QB], F32, tag="rcp")
                nc.vector.reciprocal(rcp, pv[64:65, :])
                rcpb = sb_rcp.tile([64, QB], F32, tag="rcpb")
                nc.gpsimd.partition_broadcast(rcpb, rcp)
                if h == 0:
                    dst = at01[0:64, qs]
                elif h == 1:
                    dst = at01[64:128, qs]
                else:
                    dst = at2[:, qs]
                nc.vector.tensor_mul(dst.bitcast(F32R), pv[0:64, :], rcpb)

        # ---- output projection (partial over this core's 192 channels) ---
        for co in range(C // 128):
            for tb in range(NQB):
                ts = slice(tb * QB, (tb + 1) * QB)
                po = ps_acc.tile([128, QB], F32, tag="ps_acc")
                nc.tensor.matmul(
                    po, wp0[:, co * 128 : (co + 1) * 128].bitcast(F32R),
                    at01[:, ts].bitcast(F32R), start=True, stop=False,
                )
                nc.tensor.matmul(
                    po, wp1[:, co * 128 : (co + 1) * 128].bitcast(F32R),
                    at2[:, ts].bitcast(F32R), start=False, stop=True,
                )
                ot = sb_out.tile([128, QB], F32, tag="ot")
                nc.vector.tensor_copy(ot, po)
                nc.sync.dma_start(
                    out=outT_d.ap()[co * 128 : (co + 1) * 128, ts], in_=ot
                )


_NC_CACHE = {}


def _get_nc():
    if "nc" not in _NC_CACHE:
        _NC_CACHE["nc"] = _build_nc()
    return _NC_CACHE["nc"]


def _host_consts(t_len=T):
    inv_freq = 1.0 / (ROPE_THETA ** (np.arange(0, D, 2, dtype=np.float32) / D))
    ang = np.arange(t_len, dtype=np.float32)[:, None] * inv_freq[None, :]
    sin = np.concatenate([np.sin(ang), np.sin(ang)], axis=1)  # (T, D)
    cos = np.concatenate([np.cos(ang), np.cos(ang)], axis=1)
    sinT = np.ascontiguousarray(sin.T)  # (D, T)
    cosT = np.ascontiguousarray(cos.T)
    sin2 = np.concatenate([sinT, sinT], axis=0)  # (128, T)
    cos2 = np.concatenate([cosT, cosT], axis=0)
    Z = np.zeros((D, D), dtype=np.float32)
    half = D // 2
    Z[np.arange(half), np.arange(half) + half] = 1.0   # out[m]=q[m-32], m>=32
    Z[np.arange(half) + half, np.arange(half)] = -1.0  # out[m]=-q[m+32], m<32
    p2t = np.zeros((128, 128), dtype=np.float32)
    p2t[0:D, 0:D] = Z
    p2t[D:128, D:128] = Z
    ident = np.concatenate([np.eye(D), np.eye(D)], axis=0).astype(np.float32)
    return sin2, cos2, p2t, ident


def _pack_w(w_qkv, heads):
    """Pack this core's qkv rows into the (FPAD, C) tile layout."""
    blk = {}
    for i, h in enumerate(heads):
        blk[("q", i)] = w_qkv[0 * C + h * D : 0 * C + (h + 1) * D]
        blk[("k", i)] = w_qkv[1 * C + h * D : 1 * C + (h + 1) * D]
        blk[("v", i)] = w_qkv[2 * C + h * D : 2 * C + (h + 1) * D]
    zpad = np.zeros((D, C), dtype=np.float32)
    order = [
        blk[("q", 0)], blk[("q", 1)],
        blk[("k", 0)], blk[("k", 1)],
        blk[("q", 2)], zpad,
        blk[("k", 2)], zpad,
        blk[("v", 0)], blk[("v", 1)],
        blk[("v", 2)], zpad,
    ]
    return np.concatenate(order, axis=0)  # (768, 768)


def _make_in_maps(x, w_qkv, w_proj, t_len=T):
    sin2, cos2, p2t, ident = _host_consts(t_len)
    in_maps = []
    for core in range(NCORES):
        b, hg = divmod(core, 4)
        heads = list(range(hg * HPC, (hg + 1) * HPC))
        w_sel = _pack_w(w_qkv, heads)
        cs = slice(hg * HPC * D, (hg + 1) * HPC * D)
        in_maps.append(
            {
                "xT": np.ascontiguousarray(x[b].T),
                "wT": np.ascontiguousarray(w_sel.T),
                "wpT": np.ascontiguousarray(w_proj[:, cs].T),
                "cosT": cos2, "sinT": sin2, "p2t": p2t, "ident": ident,
            }
        )
    return in_maps


def kernel(x, w_qkv, w_proj):
    x = np.asarray(x, dtype=np.float32)
    w_qkv = np.asarray(w_qkv, dtype=np.float32)
    w_proj = np.asarray(w_proj, dtype=np.float32)

    in_maps = _make_in_maps(x, w_qkv, w_proj)
    nc = _get_nc()
    res = run_bass_kernel_spmd(nc, in_maps, core_ids=list(range(NCORES)))
    out = np.zeros((B, T, C), dtype=np.float32)
    for core in range(NCORES):
        b = core // 4
        out[b] += res.results[core]["outT"].T
    return out


# revision 9
# speedup vs baseline: 11940.3018x; 11940.3018x over previous
"""Causal self-attention with RoPE, sharded over 8 TRN2 NeuronCores.

Sharding: core = (batch b, head-group hg). Cores 0-3 -> batch 0, cores 4-7 ->
batch 1; head-group hg = core % 4 owns heads [3*hg, 3*hg+3). Each core computes
its heads' attention and a partial output projection (w_proj column-slice);
the host sums the 4 partials per batch (the row-sharded projection's
all-reduce, done on host since full outputs are gathered anyway).

Per-core kernel (all fp32; matmuls run as float32r for full PE rate):
  - QKV in [feature, token] layout: out = wT.T @ xT (host pre-transposes).
    Feature tiles packed so each head's q and k share a partition half:
    ft0=[q0|q1] ft1=[k0|k1] ft2=[q2|pad] ft3=[k2|pad] ft4=[v0|v1] ft5=[v2|pad]
  - RoPE: rotate_half as a permutation matmul (p2t), combined on VectorE.
  - Attention in scores-transposed layout [keys, queries]: probs^T = exp(K^T.T
    @ Q^T * 0.125), causal mask via gpsimd affine_select (zero after exp),
    PV as out^T = (V|1).T @ probs^T accumulating over key tiles; the ones
    column yields the softmax denominator for free.
  - Division by denom folded into the PSUM->SBUF move (tensor_mul by
    partition-broadcast reciprocal).
  - Projection: partial out^T = wpT.T @ attn^T, DMA'd out as (768, 2048).
"""

import numpy as np

import concourse.bass as bass
import concourse.bacc as bacc
import concourse.tile as tile
from concourse import mybir
from concourse.bass_utils import run_bass_kernel_spmd

B, T, C, H = 2, 2048, 768, 12
D = C // H  # 64
ROPE_THETA = 10000.0
NCORES = 8
HPC = 3             # heads per core
FPAD = 768          # 6 x 128 padded qkv feature rows
QB = 512            # query block (free dim of scores^T tiles)
KT = 128            # key tile (partition dim of scores^T tiles)

F32 = mybir.dt.float32
F32R = mybir.dt.float32r

# (feature-tile, half) of each head's q / k block in the packed layout
Q_POS = {0: (0, 0), 1: (0, 1), 2: (2, 0)}
K_POS = {0: (1, 0), 1: (1, 1), 2: (3, 0)}
V_POS = {0: (4, 0), 1: (4, 1), 2: (5, 0)}


def _build_nc(t_len=T, loops=1):
    nc = bacc.Bacc("TRN2", target_bir_lowering=False, debug=False)

    xT_d = nc.dram_tensor("xT", [C, t_len], F32, kind="ExternalInput")
    wT_d = nc.dram_tensor("wT", [C, FPAD], F32, kind="ExternalInput")
    wpT_d = nc.dram_tensor("wpT", [HPC * D, C], F32, kind="ExternalInput")
    cos_d = nc.dram_tensor("cosT", [128, t_len], F32, kind="ExternalInput")
    sin_d = nc.dram_tensor("sinT", [128, t_len], F32, kind="ExternalInput")
    p2t_d = nc.dram_tensor("p2t", [128, 128], F32, kind="ExternalInput")
    id_d = nc.dram_tensor("ident", [128, D], F32, kind="ExternalInput")
    outT_d = nc.dram_tensor("outT", [C, t_len], F32, kind="ExternalOutput")

    with tile.TileContext(nc) as tc:
        _body(tc, t_len, xT_d, wT_d, wpT_d, cos_d, sin_d, p2t_d, id_d, outT_d,
              loops=loops)
    nc.compile()
    return nc


def _body(tc, t_len, xT_d, wT_d, wpT_d, cos_d, sin_d, p2t_d, id_d, outT_d,
          loops=1):
    nc = tc.nc
    T = t_len
    NQB = T // QB
    NKT = T // KT
    NCT = C // 128   # 6 contraction tiles over channels
    NFT = FPAD // 128  # 6 qkv feature tiles
    JPB = QB // KT   # key tiles per token block (4)

    with (
        tc.tile_pool(name="singles", bufs=1) as singles,
        tc.tile_pool(name="sb_x", bufs=2) as sb_x,
        tc.tile_pool(name="ps_acc", bufs=2, space="PSUM") as ps_acc,
        tc.tile_pool(name="ps_tmp", bufs=4, space="PSUM") as ps_tmp,
        tc.tile_pool(name="sb_probs", bufs=4) as sb_probs,
        tc.tile_pool(name="sb_raw", bufs=3) as sb_raw,
        tc.tile_pool(name="sb_tmp", bufs=3) as sb_tmp,
        tc.tile_pool(name="sb_out", bufs=3) as sb_out,
        tc.tile_pool(name="sb_rcp", bufs=2) as sb_rcp,
    ):
        if loops > 1:
            with tc.For_i(0, loops, 1):
                _compute(tc, nc, t_len, NQB, NKT, NCT, NFT, JPB,
                         xT_d, wT_d, wpT_d, cos_d, sin_d, p2t_d, id_d, outT_d,
                         singles, sb_x, ps_acc, ps_tmp, sb_probs, sb_raw,
                         sb_tmp, sb_out, sb_rcp)
        else:
            _compute(tc, nc, t_len, NQB, NKT, NCT, NFT, JPB,
                     xT_d, wT_d, wpT_d, cos_d, sin_d, p2t_d, id_d, outT_d,
                     singles, sb_x, ps_acc, ps_tmp, sb_probs, sb_raw,
                     sb_tmp, sb_out, sb_rcp)


def _compute(tc, nc, t_len, NQB, NKT, NCT, NFT, JPB,
             xT_d, wT_d, wpT_d, cos_d, sin_d, p2t_d, id_d, outT_d,
             singles, sb_x, ps_acc, ps_tmp, sb_probs, sb_raw,
             sb_tmp, sb_out, sb_rcp):
        T = t_len
        # ---- persistent SBUF tensors -------------------------------------
        wT = singles.tile([128, NCT, FPAD], F32, tag="wT")
        wp0 = singles.tile([128, C], F32, tag="wp0")
        wp1 = singles.tile([64, C], F32, tag="wp1")
        cosc = singles.tile([128, T], F32, tag="cosc")
        sinc = singles.tile([128, T], F32, tag="sinc")
        p2t = singles.tile([128, 128], F32, tag="p2t")
        ident = singles.tile([128, D], F32, tag="ident")
        qkrot = singles.tile([128, 4, T], F32, tag="qkrot")
        va = singles.tile([128, NKT * HPC, D + 1], F32, tag="va")
        at01 = singles.tile([128, T], F32, tag="at01")  # heads 0,1
        at2 = singles.tile([64, T], F32, tag="at2")     # head 2

        wT_v = wT_d.ap().rearrange("(a p) f -> p a f", p=128)
        for a in range(NCT):
            nc.sync.dma_start(out=wT[:, a, :].bitcast(F32R), in_=wT_v[:, a, :].bitcast(F32R))
        nc.sync.dma_start(out=wp0.bitcast(F32R), in_=wpT_d.ap()[0:128, :].bitcast(F32R))
        nc.sync.dma_start(out=wp1.bitcast(F32R), in_=wpT_d.ap()[128:192, :].bitcast(F32R))
        nc.sync.dma_start(out=cosc, in_=cos_d.ap())
        nc.sync.dma_start(out=sinc, in_=sin_d.ap())
        nc.sync.dma_start(out=p2t.bitcast(F32R), in_=p2t_d.ap().bitcast(F32R))
        nc.sync.dma_start(out=ident, in_=id_d.ap())

        # ones column of the augmented V tiles (softmax denominator trick);
        # memset can't emit fp32r, so round via a DVE copy
        ones = singles.tile([128, NKT * HPC], F32, tag="ones")
        nc.vector.memset(ones, 1.0)
        nc.vector.tensor_copy(va[:, :, D : D + 1].bitcast(F32R), ones)

        xT_v = xT_d.ap().rearrange("(a p) t -> p a t", p=128)

        # ---- QKV projection + RoPE + V transpose -------------------------
        for tb in range(NQB):
            ts = slice(tb * QB, (tb + 1) * QB)
            xtb = sb_x.tile([128, NCT, QB], F32, tag="xtb")
            for ct in range(NCT):
                nc.sync.dma_start(out=xtb[:, ct, :].bitcast(F32R), in_=xT_v[:, ct, ts].bitcast(F32R))
            for ft in range(NFT):
                acc = ps_acc.tile([128, QB], F32, tag="ps_acc")
                for ct in range(NCT):
                    nc.tensor.matmul(
                        acc,
                        wT[:, ct, ft * 128 : (ft + 1) * 128].bitcast(F32R),
                        xtb[:, ct, :].bitcast(F32R),
                        start=(ct == 0),
                        stop=(ct == NCT - 1),
                    )
                raw = sb_raw.tile([128, QB], F32, tag="raw")
                nc.vector.tensor_copy(raw.bitcast(F32R), acc)
                if ft < 4:
                    # q|k tile: rotate-half matmul, combine with sin/cos
                    rh = ps_tmp.tile([128, QB], F32, tag="ps_tmp")
                    nc.tensor.matmul(
                        rh, p2t.bitcast(F32R), raw.bitcast(F32R),
                        start=True, stop=True,
                    )
                    tmp = sb_tmp.tile([128, QB], F32, tag="tmp")
                    nc.vector.tensor_mul(tmp, rh, sinc[:, ts])
                    nc.vector.tensor_mul(
                        qkrot[:, ft, ts].bitcast(F32R), raw, cosc[:, ts]
                    )
                    nc.vector.tensor_add(
                        qkrot[:, ft, ts].bitcast(F32R), qkrot[:, ft, ts], tmp
                    )
                else:
                    # v tile: transpose each head-half into [keys, D] layout
                    for half in range(2):
                        hv = (ft - 4) * 2 + half
                        if hv >= HPC:
                            continue
                        rs = slice(half * 64, half * 64 + 64)
                        for j in range(JPB):
                            kt = tb * JPB + j
                            tp = ps_tmp.tile([128, D], F32, tag="ps_tmp")
                            nc.tensor.transpose(
                                tp,
                                raw[rs, j * KT : (j + 1) * KT],
                                ident[rs, :],
                            )
                            nc.vector.tensor_copy(
                                va[:, kt * HPC + hv, 0:D].bitcast(F32R), tp
                            )

        # ---- attention ---------------------------------------------------
        def qk_ap(pos, ts):
            ti, half = pos
            return qkrot[half * 64 : half * 64 + 64, ti, ts]

        for h in range(HPC):
            for qb in range(NQB):
                qs = slice(qb * QB, (qb + 1) * QB)
                nkt = (qb + 1) * JPB  # key tiles in causal range
                pv = ps_acc.tile([65, QB], F32, tag="ps_acc")
                for kt in range(nkt):
                    sc = ps_tmp.tile([128, QB], F32, tag="ps_tmp")
                    nc.tensor.matmul(
                        sc,
                        qk_ap(K_POS[h], slice(kt * KT, (kt + 1) * KT))
                        .bitcast(F32R),
                        qk_ap(Q_POS[h], qs).bitcast(F32R),
                        start=True, stop=True,
                    )
                    probs = sb_probs.tile([128, QB], F32, tag="probs")
                    nc.scalar.activation(
                        probs.bitcast(F32R), sc,
                        mybir.ActivationFunctionType.Exp,
                        scale=float(1.0 / np.sqrt(D)),
                    )
                    base = qb * QB - kt * KT
                    if base < KT:  # diagonal tile: zero the masked region
                        nc.gpsimd.affine_select(
                            out=probs.bitcast(F32R), in_=probs.bitcast(F32R),
                            compare_op=mybir.AluOpType.is_ge,
                            fill=0.0, base=base,
                            pattern=[[1, QB]], channel_multiplier=-1,
                        )
                    nc.tensor.matmul(
                        pv,
                        va[:, kt * HPC + h, :].bitcast(F32R),
                        probs.bitcast(F32R),
                        start=(kt == 0),
                        stop=(kt == nkt - 1),
                    )
                rcp = sb_rcp.tile([1, QB], F32, tag="rcp")
                nc.vector.reciprocal(rcp, pv[64:65, :])
                rcpb = sb_rcp.tile([64, QB], F32, tag="rcpb")
                nc.gpsimd.partition_broadcast(rcpb, rcp)
                if h == 0:
                    dst = at01[0:64, qs]
                elif h == 1:
                    dst = at01[64:128, qs]
                else:
                    dst = at2[:, qs]
                nc.vector.tensor_mul(dst.bitcast(F32R), pv[0:64, :], rcpb)

        # ---- output projection (partial over this core's 192 channels) ---
        for co in range(C // 128):
            for tb in range(NQB):
                ts = slice(tb * QB, (tb + 1) * QB)
                po = ps_acc.tile([128, QB], F32, tag="ps_acc")
                nc.tensor.matmul(
                    po, wp0[:, co * 128 : (co + 1) * 128].bitcast(F32R),
                    at01[:, ts].bitcast(F32R), start=True, stop=False,
                )
                nc.tensor.matmul(
                    po, wp1[:, co * 128 : (co + 1) * 128].bitcast(F32R),
                    at2[:, ts].bitcast(F32R), start=False, stop=True,
                )
                ot = sb_out.tile([128, QB], F32, tag="ot")
                nc.vector.tensor_copy(ot, po)
                nc.sync.dma_start(
                    out=outT_d.ap()[co * 128 : (co + 1) * 128, ts], in_=ot
                )


_NC_CACHE = {}


def _get_nc():
    if "nc" not in _NC_CACHE:
        _NC_CACHE["nc"] = _build_nc()
    return _NC_CACHE["nc"]


def _host_consts(t_len=T):
    inv_freq = 1.0 / (ROPE_THETA ** (np.arange(0, D, 2, dtype=np.float32) / D))
    ang = np.arange(t_len, dtype=np.float32)[:, None] * inv_freq[None, :]
    sin = np.concatenate([np.sin(ang), np.sin(ang)], axis=1)  # (T, D)
    cos = np.concatenate([np.cos(ang), np.cos(ang)], axis=1)
    sinT = np.ascontiguousarray(sin.T)  # (D, T)
    cosT = np.ascontiguousarray(cos.T)
    sin2 = np.concatenate([sinT, sinT], axis=0)  # (128, T)
    cos2 = np.concatenate([cosT, cosT], axis=0)
    Z = np.zeros((D, D), dtype=np.float32)
    half = D // 2
    Z[np.arange(half), np.arange(half) + half] = 1.0   # out[m]=q[m-32], m>=32
    Z[np.arange(half) + half, np.arange(half)] = -1.0  # out[m]=-q[m+32], m<32
    p2t = np.zeros((128, 128), dtype=np.float32)
    p2t[0:D, 0:D] = Z
    p2t[D:128, D:128] = Z
    ident = np.concatenate([np.eye(D), np.eye(D)], axis=0).astype(np.float32)
    return sin2, cos2, p2t, ident


def _pack_w(w_qkv, heads):
    """Pack this core's qkv rows into the (FPAD, C) tile layout."""
    blk = {}
    for i, h in enumerate(heads):
        blk[("q", i)] = w_qkv[0 * C + h * D : 0 * C + (h + 1) * D]
        blk[("k", i)] = w_qkv[1 * C + h * D : 1 * C + (h + 1) * D]
        blk[("v", i)] = w_qkv[2 * C + h * D : 2 * C + (h + 1) * D]
    zpad = np.zeros((D, C), dtype=np.float32)
    order = [
        blk[("q", 0)], blk[("q", 1)],
        blk[("k", 0)], blk[("k", 1)],
        blk[("q", 2)], zpad,
        blk[("k", 2)], zpad,
        blk[("v", 0)], blk[("v", 1)],
        blk[("v", 2)], zpad,
    ]
    return np.concatenate(order, axis=0)  # (768, 768)


def _make_in_maps(x, w_qkv, w_proj, t_len=T):
    sin2, cos2, p2t, ident = _host_consts(t_len)
    in_maps = []
    for core in range(NCORES):
        b, hg = divmod(core, 4)
        heads = list(range(hg * HPC, (hg + 1) * HPC))
        w_sel = _pack_w(w_qkv, heads)
        cs = slice(hg * HPC * D, (hg + 1) * HPC * D)
        in_maps.append(
            {
                "xT": np.ascontiguousarray(x[b].T),
                "wT": np.ascontiguousarray(w_sel.T),
                "wpT": np.ascontiguousarray(w_proj[:, cs].T),
                "cosT": cos2, "sinT": sin2, "p2t": p2t, "ident": ident,
            }
        )
    return in_maps


def kernel(x, w_qkv, w_proj):
    x = np.asarray(x, dtype=np.float32)
    w_qkv = np.asarray(w_qkv, dtype=np.float32)
    w_proj = np.asarray(w_proj, dtype=np.float32)

    in_maps = _make_in_maps(x, w_qkv, w_proj)
    nc = _get_nc()
    res = run_bass_kernel_spmd(nc, in_maps, core_ids=list(range(NCORES)))
    out = np.zeros((B, T, C), dtype=np.float32)
    for core in range(NCORES):
        b = core // 4
        out[b] += res.results[core]["outT"].T
    return out
